# revision 102
# baseline (speedup 1.0000x reference)
"""Trainium2 Bass kernel for nn_Decode (3-step Time-LSTM decoder + dense stack).

Sharding: pure data parallel over batch across 8 NeuronCores (4096 rows each),
weights replicated. Device layout is feature-major (transposed): activations
are [feature_partition, batch_free] tiles, so all weights stay PE-stationary
and batch streams as the matmul moving operand (N=512 columns per chunk = one
PSUM bank at fp32).

Host-side prep (sharding/layout only):
  - slice context_state[:, 2, :] (the model reads only step 2)
  - fold the per-step attention vectors into Wx/Wxt:
        (h*aw_t) @ W == h @ (aw_t[:,None]*W); double the g columns so that
        tanh(g) can be evaluated as 2*sigmoid(2g)-1 on the sigmoid path
  - transpose h to [HID, B]; broadcast t across partitions (replication only)

Matmul operands run bf16 (the PE streams 1 col/cycle either way; halves DMA);
sigmoid outputs and the cell chain stay fp32 — quantizing the sigmoids alone
costs 6.7e-2 rel err (the 2g'-1 reconstruction amplifies rounding ~4x) against
the 2e-2 budget.  Per step t, PSUM: gs = [i|f|o] + tgg = [Tg|g']:
  gates = Wk_t.T @ h_last (+ Uh.T @ h_t)
  o-bank  += Wto.T t            (K=1 matmul row over the broadcast t tile)
  Tg-bank += sigma(Wtt_j t_b)   (identity matmul over the ACT-computed s tile)
  ifoT[0:3] = sigmoid(gs);  ifoT[3:5] = sigmoid(tgg)      (two fused ACT ops)
  p1 = Tg*(2g'-1)  (= Tg*tanh g);  c' = f*c + i*p1;  h' = o*tanh(c')
  out_t = relu-dense x3 (h')

Schedule structure (cost-model-driven, 120.5us -> 107.4us; only op classes
proven on hardware by the f32r baseline are used — in particular GpSimd only
touches SBUF):
  - t=0 skips the f gate entirely (f*c0 == 0): 2-bank [i|o] sigmoid and two
    fewer matmuls per chunk.
  - sigma(Wtt_j t_b): t arrives pre-broadcast from the host (replication is
    layout, not compute), so the S chain is one ACT sigmoid with a
    per-partition scale — no on-device broadcast; batched 4 chunks per
    instruction at t=0 (ACT has slack there), per-chunk at t>0 (a 1.9us
    lump would stretch the tight steady-state cycle).
  - gate banks emit [o(+Wto), i, f] first: the fused [i|f|o] sigmoid of the
    next chunk is reachable after 10 matmuls while [Tg|g'] refills behind it.
  - tanh(c) and the h-multiply are emitted one chunk late so the ACT stream
    never head-of-line blocks on the DVE cell chain.
  - the dense stack is software-pipelined >= 2 chunks deep (l0 of chunk c-2,
    l1 of c-3, l2 of c-4 at chunk c), so PE never waits on a fresh relu;
    engine split: DVE p0/p2/c-add + all relus, Pool p1/p3/h multiplies
    (SBUF), ACT sigmoids/tanh only.
  - t=2's final chunk runs as two halves to shorten the serial drain; the
    fill is split across the sync and GpSimd DMA queues in need-by order
    with chunk 0's S-chain slice sigmoided as soon as the tables are warm.
"""
import sys

sys.path.insert(0, "/opt/trn_rl_repo")

import numpy as np
import concourse.bacc as bacc
import concourse.tile as tile
from concourse import mybir
from concourse.bass_utils import run_bass_kernel_spmd

N_CORES = 8
B = 32768
HID = 256
FEAT = 128
R = B // N_CORES        # batch rows per core
NB = 512                # batch columns per chunk (= one PSUM bank at fp32)
NCHUNK = R // NB
F32 = mybir.dt.float32
BF16 = mybir.dt.bfloat16
AF = mybir.ActivationFunctionType
ALU = mybir.AluOpType

DEFAULT_CFG = dict(
    zero_bias=True,   # fused sigmoids across gate banks (requires zero biases)
    split_last=True,  # taper the end of t=2 to shorten the drain
    taper=((NB // 2, NB // 2),),
    # chunks per batched sigma(Wtt t) ACT instruction, per step: big batches
    # amortize instruction overhead while ACT still has slack (t=0), singles
    # avoid stretching the tight steady-state cycles (t>0)
    squads=(4, 1, 1),
)


def build_nc(cfg=None):
    cfg = {**DEFAULT_CFG, **(cfg or {})}
    zero_bias = cfg["zero_bias"]
    squads = cfg["squads"]

    nc = bacc.Bacc(target_bir_lowering=False)

    h_d = nc.dram_tensor("h", [2, 128, R], BF16, kind="ExternalInput")
    wk_d = nc.dram_tensor("wk", [2, 128, 3, 640], BF16, kind="ExternalInput")
    uh_d = nc.dram_tensor("uh", [128, 4, 128], BF16, kind="ExternalInput")
    dw_d = nc.dram_tensor("dw", [128, 3, 128], BF16, kind="ExternalInput")
    trow_d = nc.dram_tensor("trow", [1, 2, 128], BF16, kind="ExternalInput")
    ident_d = nc.dram_tensor("ident", [128, 128], BF16, kind="ExternalInput")
    bias_d = nc.dram_tensor("bias", [128, 10], F32, kind="ExternalInput")
    # t pre-broadcast across partitions on the host (replication only)
    t_d = nc.dram_tensor("t", [128, 3, R], BF16, kind="ExternalInput")
    out_d = nc.dram_tensor("out", [3, 128, R], F32, kind="ExternalOutput")

    with tile.TileContext(nc) as tc:
        with (
            tc.tile_pool(name="const", bufs=1) as const,
            tc.tile_pool(name="act", bufs=2) as act,
            tc.tile_pool(name="ps", bufs=1, space="PSUM") as ps,
        ):
            # bias rides sync first (tiny, and the first S-chain sigmoid
            # needs its scale column; the GpSimd SWDGE path is ~1.4us slower)
            bias_sb = const.tile([128, 10], F32)
            nc.sync.dma_start(out=bias_sb[:], in_=bias_d[:])
            trow_sb = const.tile([1, 2, 128], BF16)
            nc.gpsimd.dma_start(out=trow_sb[:], in_=trow_d[:])
            ident_sb = const.tile([128, 128], BF16)
            nc.gpsimd.dma_start(out=ident_sb[:], in_=ident_d[:])
            # warm the ACT table set (sigmoid/tanh) before data arrives
            warm = const.tile([1, 2], F32)
            nc.vector.memset(warm[:], 0.0)
            nc.scalar.activation(warm[:, 0:1], warm[:, 0:1], AF.Sigmoid)
            nc.scalar.activation(warm[:, 1:2], warm[:, 1:2], AF.Tanh)
            # a dummy matmul at ~t0 starts the PE p-state ramp clock, so the
            # real matmuls (first needed at ~3.5us) start at full rate
            pe_warm = ps.tile([1, 1], F32, tag="gs", name="pe_warm")
            nc.tensor.matmul(pe_warm[:], warm[0:1, 0:1], warm[0:1, 0:1],
                             start=True, stop=True)
            wk_sb = const.tile([128, 2, 3, 640], BF16)
            hsb = const.tile([128, 2, R], BF16)
            dw_sb = const.tile([128, 3, 128], BF16)
            uh_sb = const.tile([128, 4, 128], BF16)
            wk_r = wk_d.rearrange("a p t m -> p a t m")
            h_r = h_d.rearrange("a p n -> p a n")

            # recurrent state, updated in place (the write of step t happens
            # after all step-t readers of the same column range).  h feeds
            # matmuls (bf16); c accumulates across steps and the dense stack
            # amplifies its error against the small output scale, so the c
            # chain stays fp32.
            h_st = const.tile([128, R], BF16, name="hst")
            c_st = const.tile([128, R], F32, name="cst")

            def make_insts(t):
                full = [(slice(c * NB, (c + 1) * NB), NB, f"{c}") for c in range(NCHUNK)]
                if t == 0 and cfg.get("split_first", True):
                    # head taper: half-width first chunks prime the
                    # PE<->ACT pipeline at half the round-trip latency
                    nsplit = cfg.get("head_splits", 1)
                    head = []
                    for c in range(nsplit):
                        lo = c * NB
                        head += [(slice(lo, lo + NB // 2), NB // 2, f"{c}p0"),
                                 (slice(lo + NB // 2, lo + NB), NB // 2, f"{c}p1")]
                    full = head + full[nsplit:]
                if t == 2 and cfg.get("split_last", False):
                    # taper the trailing chunks to shorten the serial drain:
                    # each entry of cfg["taper"] re-splits one trailing chunk
                    taper = cfg.get("taper", ((NB // 2, NB // 2),) * 2)
                    out = full[:len(full) - len(taper)]
                    c = NCHUNK - len(taper)
                    for parts in taper:
                        lo = c * NB
                        for pi, w in enumerate(parts):
                            out.append((slice(lo, lo + w), w, f"{c}p{pi}"))
                            lo += w
                        c += 1
                    return out
                return full

            # ---- batched S chain: sigma(Wtt_j * t_b) ----
            # sgroups[g] = (t, lo, hi): one ACT sigmoid covers chunks
            # [lo, hi); scheduled 2 blocks before first use, t tile 2 earlier
            sgroups = []
            for t in range(3):
                w = squads[t] * NB
                for lo in range(0, R, w):
                    sgroups.append((t, lo, min(lo + w, R)))
            inst_sg = {}      # (t, key) -> (gidx, offset within group)
            need_block = {}   # gidx -> first global block using it
            gb_of = {}
            gb = 0
            for t in range(3):
                for col, nb, key in make_insts(t):
                    g = next(i for i, (tt_, lo, hi) in enumerate(sgroups)
                             if tt_ == t and lo <= col.start < hi)
                    inst_sg[(t, key)] = (g, col.start - sgroups[g][1])
                    need_block.setdefault(g, gb)
                    gb_of[(t, key)] = gb
                    gb += 1
            n_blocks = gb
            # groups needed by block <= 2 are loaded + sigmoided in the fill
            # section; the rest trigger inside the block loop, 2 blocks ahead
            n_head = sum(1 for g in need_block if need_block[g] <= 2)
            emit_at = {g: max(1, need_block[g] - 2) for g in need_block}
            load_at = {g: max(1, emit_at[g] - 2) for g in need_block}

            t_tiles = {}     # gidx -> broadcast t tile
            s_tiles = {}     # gidx -> sigma values

            def load_t(g):
                t, lo, hi = sgroups[g]
                tt = act.tile([128, hi - lo], BF16, tag="t_tile", bufs=4,
                              name=f"tt_{g}")
                nc.sync.dma_start(out=tt[:], in_=t_d[:, t, lo:hi])
                t_tiles[g] = tt

            def emit_schain(g):
                tt = t_tiles[g]
                s_sb = act.tile([128, tt.shape[-1]], BF16, tag="s_sb", bufs=3,
                                name=f"s_{g}")
                nc.scalar.activation(s_sb[:], tt[:], AF.Sigmoid,
                                     scale=bias_sb[:, 8:9])
                s_tiles[g] = s_sb

            # fill: split the first-needed tiles across the sync and GpSimd
            # DMA queues so the o/i banks of chunk 0 are matmul-ready ASAP.
            # t=0 bank order is o(m2,+Wto), i(m0), Tg(m3,+ident), g(m4).
            # The first S chain runs in two pieces: chunk 0's slice as soon
            # as the tables are warm, the rest behind the first sigmoids.
            t0g, lo0, hi0 = sgroups[0]
            tt0 = act.tile([128, hi0 - lo0], BF16, tag="t_tile", bufs=4,
                           name="tt_0")
            nc.sync.dma_start(out=tt0[:, 0:NB], in_=t_d[:, t0g, lo0:lo0 + NB])
            # h chunk 0 in halves: the head-taper block 0 only reads the
            # first 256 columns, so its gates start one half-DMA earlier
            nc.sync.dma_start(out=hsb[:, :, 0:NB // 2], in_=h_r[:, :, 0:NB // 2])
            nc.sync.dma_start(out=hsb[:, :, NB // 2:NB], in_=h_r[:, :, NB // 2:NB])
            for m in (2, 0):
                nc.gpsimd.dma_start(out=wk_sb[:, :, 0, m * 128:(m + 1) * 128],
                                    in_=wk_r[:, :, 0, m * 128:(m + 1) * 128])
            s0 = act.tile([128, hi0 - lo0], BF16, tag="s_sb", bufs=3, name="s_0")
            nc.scalar.activation(s0[:, 0:NB], tt0[:, 0:NB], AF.Sigmoid,
                                 scale=bias_sb[:, 8:9])
            if hi0 - lo0 > NB:
                nc.sync.dma_start(out=tt0[:, NB:], in_=t_d[:, t0g, lo0 + NB:hi0])
            for m in (3, 4, 1):
                nc.sync.dma_start(out=wk_sb[:, :, 0, m * 128:(m + 1) * 128],
                                  in_=wk_r[:, :, 0, m * 128:(m + 1) * 128])
            # the rest of group 0's sigmoid is deferred into block 0's ACT
            # stream so it doesn't head-of-line block chunk 0's gate sigmoid
            def s0_rest():
                if hi0 - lo0 > NB:
                    nc.scalar.activation(s0[:, NB:], tt0[:, NB:], AF.Sigmoid,
                                         scale=bias_sb[:, 8:9])
            t_tiles[0] = tt0
            s_tiles[0] = s0
            # remaining head groups (+ one lookahead), h1 wedged in between
            # so block 1's gates are not starved
            load_t(1)
            nc.sync.dma_start(out=hsb[:, :, NB:2 * NB], in_=h_r[:, :, NB:2 * NB])
            for g in range(2, n_head + 1):
                if g < len(sgroups):
                    load_t(g)
            for g in range(1, n_head):
                emit_schain(g)
            nc.sync.dma_start(out=dw_sb[:], in_=dw_d[:])
            # all h chunks before the (bulky) later-step weights: t=0 eats a
            # chunk every ~3.2us while wk[t1]/uh are only needed at ~30us
            for c in range(2, NCHUNK):
                col = slice(c * NB, (c + 1) * NB)
                nc.sync.dma_start(out=hsb[:, :, col], in_=h_r[:, :, col])
                if c == 5:
                    nc.sync.dma_start(out=uh_sb[:], in_=uh_d[:])
                if c == 6:
                    nc.sync.dma_start(out=wk_sb[:, :, 1, :], in_=wk_r[:, :, 1, :])
                if c == 7:
                    nc.sync.dma_start(out=wk_sb[:, :, 2, :], in_=wk_r[:, :, 2, :])
            sg_load = [n_head + 1]
            sg_emit = [n_head]

            def emit_gates(t, inst, h_prev):
                """PE gate matmuls + GpSimd PSUM injections for one chunk.

                gs = [i|f|o] (t=0: [i|o]) with o emitted FIRST so its GpSimd
                Wto-injection hides behind the i/f matmuls; tgg = [Tg|g']
                with Tg first for the same reason."""
                col, nb, key = inst
                g, qoff = inst_sg[(t, key)]
                t_sl = t_tiles[g][:, qoff:qoff + nb]
                s_sl = s_tiles[g][:, qoff:qoff + nb]
                nbank = 3 if t > 0 else 2
                gs = ps.tile([128, nbank, nb], F32, tag="gs", name=f"gs_{key}_{t}")
                tgg = ps.tile([128, 2, nb], F32, tag="tgg", name=f"tgg_{key}_{t}")
                o_slot = nbank - 1

                def bank(tgt, m, uh_row, extra=None):
                    n_ex = (1 if (t > 0 and uh_row is not None) else 0) \
                        + (1 if extra is not None else 0)
                    for k in range(2):
                        nc.tensor.matmul(
                            tgt, wk_sb[:, k, t, m * 128:(m + 1) * 128],
                            hsb[:, k, col],
                            start=(k == 0), stop=(k == 1 and n_ex == 0),
                        )
                    if t > 0 and uh_row is not None:
                        n_ex -= 1
                        nc.tensor.matmul(
                            tgt, uh_sb[:, uh_row, :], h_prev[:, col],
                            start=False, stop=(n_ex == 0),
                        )
                    if extra is not None:
                        extra(tgt, True)

                # gs banks first ([o] then i, f) so the fused [i|f|o] sigmoid
                # is reachable after 10 matmuls; the Wto term accumulates
                # inside the o group (K=1 row of the broadcast t tile)
                def wto(tgt, last):
                    nc.tensor.matmul(tgt, trow_sb[:, 1, :], t_sl[0:1, :],
                                     start=False, stop=last)

                def ident(tgt, last):
                    nc.tensor.matmul(tgt, ident_sb[:], s_sl,
                                     start=False, stop=last)

                bank(gs[:, o_slot, :], 2, 2, extra=wto)
                bank(gs[:, 0, :], 0, 0)
                if t > 0:
                    bank(gs[:, 1, :], 1, 1)
                # tgg banks refill during the [i|f|o] sigmoid
                bank(tgg[:, 0, :], 3, None, extra=ident)
                bank(tgg[:, 1, :], 4, 3)
                return gs, tgg

            def emit_sigmas(t, gs, tgg, ifoT_dst):
                """fused sigmoids: [i|f|o] -> slots 0:3, [Tg|g'] -> slots 3:5
                (t=0: [i|o] -> 0:2, [Tg|g'] -> 2:4)."""
                nbank = 3 if t > 0 else 2
                if zero_bias:
                    nc.scalar.activation(ifoT_dst[:, 0:nbank, :], gs[:], AF.Sigmoid)
                    nc.scalar.activation(ifoT_dst[:, nbank:nbank + 2, :], tgg[:],
                                         AF.Sigmoid)
                else:
                    srcs = [gs[:, j, :] for j in range(nbank)] \
                        + [tgg[:, 0, :], tgg[:, 1, :]]
                    bidx = ([0, 1, 2] if t > 0 else [0, 2]) + [3, 4]
                    for j in range(nbank + 2):
                        nc.scalar.activation(ifoT_dst[:, j, :], srcs[j],
                                             AF.Sigmoid,
                                             bias=bias_sb[:, bidx[j]:bidx[j] + 1])

            # ---- software-pipelined dense stack ----
            # l0 of chunk c-2, l1 of c-3, l2 of c-4 are emitted at chunk c's
            # block head, so every dense dependency is >= 2 chunks old and
            # neither PE nor Pool ever head-of-line blocks on fresh results.
            dense_pend = []   # items: [t, col, nb, key, stage, cur_tile, idx]

            def advance_dense(bidx):
                for it in [x for x in dense_pend if x[4] == 2] \
                        + [x for x in dense_pend if x[4] == 1] \
                        + [x for x in dense_pend if x[4] == 0]:
                    t, col, nb, key, l, cur, idx = it
                    if l == 0 and idx > bidx - 2:
                        continue
                    dps = ps.tile([128, nb], F32, tag="dps", bufs=3,
                                  name=f"dps_{key}_{t}_{l}")
                    nc.tensor.matmul(
                        dps[:], dw_sb[:, l, :],
                        h_st[:, col] if l == 0 else cur[:],
                        start=True, stop=True,
                    )
                    odt = F32 if l == 2 else BF16
                    dsb = act.tile([128, nb], odt, tag=f"dsb{l}", bufs=3,
                                   name=f"d_{key}_{t}_{l}")
                    if not zero_bias:
                        nc.scalar.activation(
                            dsb[:], dps[:], AF.Relu, bias=bias_sb[:, 5 + l:6 + l]
                        )
                    else:
                        nc.vector.tensor_relu(dsb[:], dps[:])
                    it[4] += 1
                    it[5] = dsb
                    if l == 2:
                        nc.sync.dma_start(out=out_d[t, :, col], in_=dsb[:])
                dense_pend[:] = [x for x in dense_pend if x[4] < 3]

            # ---- lagged tanh(c) + h-multiply (one chunk late) ----
            tanh_pend = []    # items: (t, col, nb, key, o_slice)

            def flush_tanh():
                for t, col, nb, key, o_sl in tanh_pend:
                    tanh_c = act.tile([128, nb], BF16, tag="tanh_c",
                                      name=f"tc_{key}_{t}")
                    nc.scalar.activation(tanh_c[:], c_st[:, col], AF.Tanh)
                    nc.gpsimd.tensor_mul(h_st[:, col], o_sl, tanh_c[:])
                tanh_pend.clear()

            for t in range(3):
                for ii, inst in enumerate(make_insts(t)):
                    col, nb, key = inst
                    gb = gb_of[(t, key)]
                    while sg_load[0] < len(sgroups) and \
                            load_at[sg_load[0]] <= gb:
                        load_t(sg_load[0])
                        sg_load[0] += 1
                    gs, tgg = emit_gates(t, inst, h_st)
                    # slots: t>0 [i,f,o,Tg,g']; t=0 [i,o,Tg,g'].  fp32: bf16
                    # sigmoid outputs alone cost 6.7e-2 rel err (the 2g'-1
                    # reconstruction amplifies quantization by ~4x)
                    nslot = 5 if t > 0 else 4
                    ifoT = act.tile([128, nslot, nb], F32, tag="ifoT", bufs=3,
                                    name=f"ifoT_{key}_{t}")
                    emit_sigmas(t, gs, tgg, ifoT[:])
                    # ACT filler work while PE refills the gate banks:
                    # upcoming S chains + the lagged tanh(c)/h of chunk c-1
                    if gb == 0:
                        s0_rest()
                    while sg_emit[0] < len(sgroups) and \
                            emit_at[sg_emit[0]] <= gb:
                        emit_schain(sg_emit[0])
                        sg_emit[0] += 1
                    flush_tanh()
                    i_g = ifoT[:, 0, :]
                    f_g = ifoT[:, 1, :] if t > 0 else None
                    o_g = ifoT[:, nslot - 3, :]
                    tg_g = ifoT[:, nslot - 2, :]
                    gp_g = ifoT[:, nslot - 1, :]

                    # ---- cell chain: p1 = Tg*tanh(g) = Tg*(2g'-1); the
                    # multiplies run on GpSimd (SBUF only), the rest on DVE.
                    p0 = act.tile([128, nb], F32, tag="p0", name=f"p0_{key}_{t}")
                    nc.vector.tensor_scalar(
                        out=p0[:], in0=gp_g, scalar1=2.0, scalar2=1.0,
                        op0=ALU.mult, op1=ALU.subtract)
                    p1 = act.tile([128, nb], BF16, tag="p1", name=f"p1_{key}_{t}")
                    nc.gpsimd.tensor_mul(p1[:], tg_g, p0[:])
                    if t == 0:
                        nc.vector.tensor_mul(c_st[:, col], i_g, p1[:])
                    else:
                        p2 = act.tile([128, nb], F32, tag="p2", name=f"p2_{key}_{t}")
                        nc.vector.tensor_mul(p2[:], i_g, p1[:])
                        p3 = act.tile([128, nb], F32, tag="p3", name=f"p3_{key}_{t}")
                        nc.gpsimd.tensor_mul(p3[:], f_g, c_st[:, col])
                        nc.vector.tensor_add(c_st[:, col], p2[:], p3[:])

                    # dense stages of older chunks (PE slots after this
                    # chunk's gate matmuls), then queue this chunk
                    advance_dense(gb)
                    tanh_pend.append((t, col, nb, key, o_g))
                    dense_pend.append([t, col, nb, key, 0, None, gb])

            # drain: last tanh/h, then the dense pipeline
            flush_tanh()
            for _ in range(4):
                advance_dense(n_blocks + 4)

    nc.finalize()
    return nc


_NC_CACHE = {}


def _get_nc(key, cfg):
    if key not in _NC_CACHE:
        _NC_CACHE[key] = build_nc(cfg)
    return _NC_CACHE[key]


def _to_bf16(a):
    import ml_dtypes
    return np.asarray(a, np.float32).astype(ml_dtypes.bfloat16)


def kernel(context_state, input_t, aw1, aw2, aw3, Wx, Uh, b,
           Wxt, Wtt, bt, Wto, w1, b1, w2, b2, w3, b3):
    f32 = np.float32
    f64 = np.float64

    # ---- host-side prep / sharding ----
    h_last = np.asarray(context_state)[:, 2, :].astype(f32)          # [B, HID]
    hT = np.ascontiguousarray(h_last.T).reshape(2, 128, B)           # [2,128,B]
    tT = np.ascontiguousarray(np.asarray(input_t)[:, 3:, 0].T)       # [3, B]
    aw = np.concatenate(
        [np.asarray(aw1), np.asarray(aw2), np.asarray(aw3)], axis=1
    )[0].astype(f64)                                                 # [3, HID]

    Wx64, Wxt64 = np.asarray(Wx, f64), np.asarray(Wxt, f64)
    wk = np.empty((HID, 3, 640), f64)
    for t in range(3):
        wxf = aw[t][:, None] * Wx64                                  # [HID, 512]
        wtf = aw[t][:, None] * Wxt64                                 # [HID, 128]
        wk[:, t, 0:384] = wxf[:, 0:384]          # i, f, o
        wk[:, t, 384:512] = wtf                  # Tg
        wk[:, t, 512:640] = 2.0 * wxf[:, 384:512]  # g (doubled: tanh via 2s-1)
    wk = _to_bf16(wk.astype(f32)).reshape(2, 128, 3, 640)

    uh4 = np.asarray(Uh, f32).reshape(128, 4, 128).copy()
    uh4[:, 3, :] *= 2.0                          # g row doubled as well
    uh = _to_bf16(uh4)
    dw = _to_bf16(np.stack(
        [np.asarray(w1, f32), np.asarray(w2, f32), np.asarray(w3, f32)], axis=1))
    trow = _to_bf16(
        np.stack([np.asarray(Wtt, f32)[0], np.asarray(Wto, f32)[0]], axis=0)
    ).reshape(1, 2, 128)
    ident = _to_bf16(np.eye(128, dtype=f32))
    bias = np.ascontiguousarray(np.stack(
        [np.asarray(b, f32)[0:128], np.asarray(b, f32)[128:256],
         np.asarray(b, f32)[256:384], np.asarray(bt, f32),
         2.0 * np.asarray(b, f32)[384:512],   # g bias doubled (tanh = 2s-1)
         np.asarray(b1, f32),
         np.asarray(b2, f32), np.asarray(b3, f32),
         np.asarray(Wtt, f32)[0], np.asarray(Wto, f32)[0]], axis=1))  # [128,10]

    zero_bias = not (bias[:, 0:8].any())
    cfg = dict(DEFAULT_CFG, zero_bias=zero_bias)
    nc = _get_nc(("main", zero_bias), cfg)

    hT16 = _to_bf16(hT)
    tT16 = _to_bf16(tT)
    in_maps = []
    for core in range(N_CORES):
        rs = slice(core * R, (core + 1) * R)
        t_core = np.ascontiguousarray(
            np.broadcast_to(tT16[None, :, rs], (128, 3, R)))
        in_maps.append(dict(
            h=np.ascontiguousarray(hT16[:, :, rs]),
            wk=wk, uh=uh, dw=dw, trow=trow, ident=ident, bias=bias,
            t=t_core,
        ))

    global _LAST_IN_MAPS
    _LAST_IN_MAPS = in_maps
    res = run_bass_kernel_spmd(nc, in_maps, core_ids=list(range(N_CORES)))
    outs = [np.transpose(res.results[c]["out"], (2, 0, 1)) for c in range(N_CORES)]
    return np.ascontiguousarray(np.concatenate(outs, axis=0))
